# revision 3
# baseline (speedup 1.0000x reference)
"""Causal self-attention (B=2, T=2048, C=1024, H=16) on 8 TRN2 NeuronCores.

Sharding: data parallel over batch (2) x tensor parallel over heads (4 groups
of 4 heads). Each core computes qkv + attention for its 4 heads of one batch;
normalized attention outputs are AllGathered (per t-chunk, per head pair)
within each batch group of 4 cores, and each core then computes a 256-column
slice of the output projection. The host concatenates the column slices.

v2: qkv for all chunks up front, attention chunks processed in reverse order
(largest first) so the final AllGather covers the smallest chunk; per-head
software-pipelined QK->exp->AV loop with fine-grained causal trimming.
"""
import numpy as np
import ml_dtypes

import concourse.bass as bass
import concourse.tile as tile
from concourse import bacc, mybir
from concourse.bass_utils import run_bass_kernel_spmd

BF16 = ml_dtypes.bfloat16

B, T, C, H, D = 2, 2048, 1024, 16, 64
NCORES = 8
HPC = 4              # heads per core
FQK = 2 * HPC * D    # 512 rows of q+k per core
FV = HPC * D         # 256 rows of v per core
CT = C // 128        # 8 contraction tiles
TC5 = T // 512       # 4 t-chunks of 512
SB = T // 128        # 16 s-blocks of 128
SCALE = 1.0 / 8.0    # 1/sqrt(D)

_CACHE = {}


def _build_kernel():
    nc = bacc.Bacc("TRN2", target_bir_lowering=False, debug=False,
                   num_devices=NCORES)
    dt = mybir.dt
    f32, bf16 = dt.float32, dt.bfloat16

    xT = nc.dram_tensor("xT", [C, T], bf16, kind="ExternalInput").ap()
    wqkT = nc.dram_tensor("wqkT", [C, FQK], bf16, kind="ExternalInput").ap()
    wvT = nc.dram_tensor("wvT", [C, FV], bf16, kind="ExternalInput").ap()
    wpT = nc.dram_tensor("wpT", [C, FV], bf16, kind="ExternalInput").ap()
    bqk = nc.dram_tensor("bqk", [FQK, 1], f32, kind="ExternalInput").ap()
    bv = nc.dram_tensor("bv", [1, FV], bf16, kind="ExternalInput").ap()
    bp = nc.dram_tensor("bp", [1, FV], bf16, kind="ExternalInput").ap()
    maskb = nc.dram_tensor("maskb", [128, 128], bf16, kind="ExternalInput").ap()
    out = nc.dram_tensor("out", [T, FV], f32, kind="ExternalOutput").ap()

    xTr = xT.rearrange("(n p) t -> p n t", p=128)
    wqkr = wqkT.rearrange("(n p) f -> p n f", p=128)
    wvr = wvT.rearrange("(n p) f -> p n f", p=128)
    wpr = wpT.rearrange("(n p) f -> p n f", p=128)

    with tile.TileContext(nc) as tc:
        with (
            tc.tile_pool(name="persist", bufs=1) as pp,
            tc.tile_pool(name="work", bufs=4) as wp,
            tc.tile_pool(name="attT", bufs=6) as ap_pool,
            tc.tile_pool(name="outsb", bufs=3) as op,
            tc.tile_pool(name="ps_s", bufs=4, space="PSUM") as ps_s,
            tc.tile_pool(name="ps_y", bufs=2, space="PSUM") as ps_y,
            tc.tile_pool(name="ps_mm", bufs=2, space="PSUM") as ps_mm,
            tc.tile_pool(name="dram", bufs=1, space="DRAM") as dram,
        ):
            # ---- DMA loads, ordered so qkT(0)'s operands land first ----
            bqk_s = pp.tile([128, 4], f32, tag="bqk")
            nc.sync.dma_start(bqk_s[:], bqk.rearrange("(n p) o -> p (n o)", p=128))
            mask_s = pp.tile([128, 128], bf16, tag="mask")
            nc.sync.dma_start(mask_s[:], maskb[:])

            wqk_s = pp.tile([128, CT, FQK], bf16, tag="wqk")
            xT_s = pp.tile([128, CT, T], bf16, tag="xT")
            wv_s = pp.tile([128, CT, FV], bf16, tag="wv")
            wp_s = pp.tile([128, CT, FV], bf16, tag="wp")
            for ci in range(CT):
                nc.sync.dma_start(wqk_s[:, ci, :], wqkr[:, ci, :])
                nc.sync.dma_start(xT_s[:, ci, 0:512], xTr[:, ci, 0:512])
            for t5x in range(1, TC5):
                for ci in range(CT):
                    nc.sync.dma_start(xT_s[:, ci, t5x * 512:(t5x + 1) * 512],
                                      xTr[:, ci, t5x * 512:(t5x + 1) * 512])
            for ci in range(CT):
                nc.sync.dma_start(wv_s[:, ci, :], wvr[:, ci, :])
            nc.sync.dma_start(wp_s[:], wpr[:])
            bv_s = pp.tile([1, FV], bf16, tag="bv")
            nc.sync.dma_start(bv_s[:], bv[:])
            bp_s = pp.tile([1, FV], bf16, tag="bp")
            nc.sync.dma_start(bp_s[:], bp[:])

            ones16 = pp.tile([1, 128], bf16, tag="ones16")
            nc.vector.memset(ones16[:], 1.0)

            # ---- qkv producers ----
            qkT_s = [pp.tile([128, T], bf16, tag=f"qkT{fc}", name=f"qkT{fc}")
                     for fc in range(4)]
            vaug = pp.tile([128, SB, HPC * 65], bf16, tag="vaug")
            nc.vector.memset(vaug[:], 1.0)
            bv_bc = pp.tile([128, FV], f32, tag="bv_bc")
            bp_bc = pp.tile([128, FV], f32, tag="bp_bc")

            def qkT_chunk(t5):
                for fc in range(4):
                    ps = ps_s.tile([128, 512], f32, tag="s", name="ps_qkv")
                    for ci in range(CT):
                        nc.tensor.matmul(
                            ps[:],
                            wqk_s[:, ci, fc * 128:(fc + 1) * 128],
                            xT_s[:, ci, t5 * 512:(t5 + 1) * 512],
                            start=(ci == 0), stop=(ci == CT - 1),
                        )
                    nc.vector.tensor_scalar_add(
                        qkT_s[fc][:, t5 * 512:(t5 + 1) * 512], ps[:],
                        bqk_s[:, fc:fc + 1],
                    )

            def bias_bc(row, bc_t):
                psb = ps_mm.tile([128, FV], f32, tag="mm", name="ps_bias")
                nc.tensor.matmul(psb[:], ones16[0:1, :], row[0:1, :],
                                 start=True, stop=True)
                nc.vector.tensor_copy(bc_t[:], psb[:])

            def v_chunk(t5):
                for tb in range(4 * t5, 4 * t5 + 4):
                    ps = ps_mm.tile([128, FV], f32, tag="mm", name="ps_v")
                    for ci in range(CT):
                        nc.tensor.matmul(
                            ps[:],
                            xT_s[:, ci, tb * 128:(tb + 1) * 128],
                            wv_s[:, ci, :],
                            start=(ci == 0), stop=(ci == CT - 1),
                        )
                    dst = vaug[:, tb, :].rearrange("p (h x) -> p h x", h=HPC)[:, :, 0:64]
                    src = ps[:].rearrange("p (h x) -> p h x", h=HPC)
                    bias = bv_bc[:].rearrange("p (h x) -> p h x", h=HPC)
                    nc.vector.scalar_tensor_tensor(
                        dst, src, 1.0, bias,
                        op0=mybir.AluOpType.mult, op1=mybir.AluOpType.add,
                    )

            # ---- attention: per-head pipelined QK -> exp -> AV ----
            ag_in, ag_out, yf = {}, {}, {}
            for t5 in range(TC5):
                for pr in range(2):
                    ag_in[(t5, pr)] = dram.tile([128, 512], bf16,
                                                tag=f"agin{t5}_{pr}",
                                                name=f"agin{t5}_{pr}")
                    ag_out[(t5, pr)] = dram.tile([512, 512], bf16,
                                                 tag=f"agout{t5}_{pr}",
                                                 name=f"agout{t5}_{pr}")

            def att_chunk(t5):
                live = 4 * (t5 + 1)
                yf[t5] = pp.tile([128, 2, CT // 2, 512], bf16, tag="yf", bufs=2,
                                 name=f"yf{t5}")
                for head in range(HPC):
                    pair, hh = head // 2, head % 2
                    q_fc, k_fc = pair, 2 + pair
                    lo, hi = 64 * hh, 64 * (hh + 1)
                    yps = ps_y.tile([65, 512], f32, tag="y", name="yps")
                    a_t = [None] * live
                    qlo = [max(0, sb * 128 - t5 * 512) for sb in range(live)]

                    def qk_block(sb):
                        ps = ps_s.tile([128, 512], f32, tag="s", name="ps_s")
                        nc.tensor.matmul(
                            ps[:, qlo[sb]:512],
                            qkT_s[k_fc][lo:hi, sb * 128:(sb + 1) * 128],
                            qkT_s[q_fc][lo:hi,
                                        t5 * 512 + qlo[sb]:(t5 + 1) * 512],
                            start=True, stop=True,
                        )
                        a = ap_pool.tile([128, 512], bf16, tag="attT",
                                         name="attT")
                        nc.scalar.activation(
                            a[:, qlo[sb]:512], ps[:, qlo[sb]:512],
                            mybir.ActivationFunctionType.Exp, scale=SCALE,
                        )
                        off = sb * 128 - t5 * 512
                        if off >= 0:
                            nc.vector.tensor_mul(a[:, off:off + 128],
                                                 a[:, off:off + 128], mask_s[:])
                        a_t[sb] = a

                    def av_block(sb):
                        nc.tensor.matmul(
                            yps[:, qlo[sb]:512],
                            vaug[:, sb, head * 65:(head + 1) * 65],
                            a_t[sb][:, qlo[sb]:512],
                            start=(sb == 0), stop=(sb == live - 1),
                            skip_group_check=True,
                        )

                    qk_block(0)
                    qk_block(1)
                    for sb in range(2, live):
                        qk_block(sb)
                        av_block(sb - 2)
                    av_block(live - 2)
                    av_block(live - 1)

                    # normalize: y / denom (denom accumulated via ones row 64)
                    den = wp.tile([1, 512], bf16, tag="den", name="den")
                    nc.vector.tensor_copy(den[:], yps[64:65, :])
                    bc = ps_s.tile([128, 512], f32, tag="s", name="bc")
                    nc.tensor.matmul(bc[0:64, :], ones16[0:1, 0:64], den[:],
                                     start=True, stop=True)
                    r = wp.tile([64, 512], f32, tag="recip", name="recip")
                    nc.vector.reciprocal_approx_fast(r[:], bc[0:64, :])
                    yn = wp.tile([64, 512], bf16, tag="yn", name="yn")
                    nc.vector.tensor_mul(yn[:], yps[0:64, :], r[:])
                    nc.sync.dma_start(
                        ag_in[(t5, pair)][hh * 64:(hh + 1) * 64, :], yn[:])
                    if hh == 1:
                        nc.gpsimd.collective_compute(
                            "AllGather", mybir.AluOpType.bypass,
                            replica_groups=[[0, 1, 2, 3], [4, 5, 6, 7]],
                            ins=[ag_in[(t5, pair)][:].opt()],
                            outs=[ag_out[(t5, pair)][:].opt()],
                        )
                        nc.gpsimd.dma_start(
                            yf[t5][:, pair, :, :],
                            ag_out[(t5, pair)][:].rearrange(
                                "(n p) t -> p n t", p=128))

            def proj_chunk(t5):
                yft = yf[t5]
                for tq in range(4):
                    tb = t5 * 4 + tq
                    pso = ps_mm.tile([128, FV], f32, tag="mm", name="ps_o")
                    for ci in range(CT):
                        par, cc = ci % 2, ci // 2
                        nc.tensor.matmul(
                            pso[:],
                            yft[:, par, cc, tq * 128:(tq + 1) * 128],
                            wp_s[:, ci, :],
                            start=(ci == 0), stop=(ci == CT - 1),
                        )
                    osb = op.tile([128, FV], f32, tag="osb", name="osb")
                    nc.vector.tensor_add(osb[:], pso[:], bp_bc[:])
                    nc.sync.dma_start(out[tb * 128:(tb + 1) * 128, :], osb[:])

            # ---- schedule ----
            for t5 in range(TC5):
                qkT_chunk(t5)
            bias_bc(bv_s, bv_bc)
            bias_bc(bp_s, bp_bc)
            for t5 in range(TC5):
                v_chunk(t5)
            att_chunk(3)
            att_chunk(2)
            proj_chunk(3)
            att_chunk(1)
            proj_chunk(2)
            att_chunk(0)
            proj_chunk(1)
            proj_chunk(0)

    nc.compile()
    return nc


def _shard_inputs(x, w_attn, b_attn, w_proj, b_proj):
    mask = np.zeros((128, 128), dtype=BF16)
    for p in range(128):
        mask[p, p:] = 1.0

    in_maps = []
    for core in range(NCORES):
        b, hg = core // 4, core % 4
        r0 = hg * HPC * D          # first q/k/v row offset within each 1024
        r1 = r0 + HPC * D
        wqk = np.concatenate([w_attn[r0:r1, :], w_attn[C + r0:C + r1, :]], 0)
        in_maps.append({
            "xT": np.ascontiguousarray(x[b].T).astype(BF16),
            "wqkT": np.ascontiguousarray(wqk.T).astype(BF16),
            "wvT": np.ascontiguousarray(w_attn[2 * C + r0:2 * C + r1, :].T).astype(BF16),
            "wpT": np.ascontiguousarray(w_proj[r0:r1, :].T).astype(BF16),
            "bqk": np.concatenate([b_attn[r0:r1], b_attn[C + r0:C + r1]])
                     .reshape(FQK, 1).astype(np.float32),
            "bv": b_attn[2 * C + r0:2 * C + r1].reshape(1, FV).astype(BF16),
            "bp": b_proj[r0:r1].reshape(1, FV).astype(BF16),
            "maskb": mask,
        })
    return in_maps


def kernel(x, w_attn, b_attn, w_proj, b_proj, _trace=False, _trace_kwargs=None):
    x = np.asarray(x, dtype=np.float32)
    w_attn = np.asarray(w_attn, dtype=np.float32)
    b_attn = np.asarray(b_attn, dtype=np.float32)
    w_proj = np.asarray(w_proj, dtype=np.float32)
    b_proj = np.asarray(b_proj, dtype=np.float32)

    if "nc" not in _CACHE:
        _CACHE["nc"] = _build_kernel()
    nc = _CACHE["nc"]

    in_maps = _shard_inputs(x, w_attn, b_attn, w_proj, b_proj)
    res = run_bass_kernel_spmd(nc, in_maps, core_ids=list(range(NCORES)),
                               trace=_trace, **(_trace_kwargs or {}))
    _CACHE["last_result"] = res

    out = np.empty((B, T, C), dtype=np.float32)
    for core in range(NCORES):
        b, hg = core // 4, core % 4
        out[b, :, hg * FV:(hg + 1) * FV] = res.results[core]["out"]
    return out


# revision 8
# speedup vs baseline: 1.1896x; 1.1896x over previous
"""Causal self-attention (B=2, T=2048, C=1024, H=16) on 8 TRN2 NeuronCores.

Sharding: data parallel over batch (2) x tensor parallel over heads (4 groups
of 4 heads). Each core computes qkv + attention for its 4 heads of one batch;
normalized attention outputs are AllGathered (per t-chunk, per head pair)
within each batch group of 4 cores, and each core then computes a 256-column
slice of the output projection. The host concatenates the column slices.

v2: qkv for all chunks up front, attention chunks processed in reverse order
(largest first) so the final AllGather covers the smallest chunk; per-head
software-pipelined QK->exp->AV loop with fine-grained causal trimming.
"""
import numpy as np
import ml_dtypes

import concourse.bass as bass
import concourse.tile as tile
from concourse import bacc, mybir
from concourse.bass_utils import run_bass_kernel_spmd

BF16 = ml_dtypes.bfloat16

B, T, C, H, D = 2, 2048, 1024, 16, 64
NCORES = 8
HPC = 4              # heads per core
FQK = 2 * HPC * D    # 512 rows of q+k per core
FV = HPC * D         # 256 rows of v per core
CT = C // 128        # 8 contraction tiles
TC5 = T // 512       # 4 t-chunks of 512
SB = T // 128        # 16 s-blocks of 128
SCALE = 1.0 / 8.0    # 1/sqrt(D)

_CACHE = {}


def _build_kernel():
    nc = bacc.Bacc("TRN2", target_bir_lowering=False, debug=False,
                   num_devices=NCORES)
    dt = mybir.dt
    f32, bf16 = dt.float32, dt.bfloat16

    xT = nc.dram_tensor("xT", [C, T], bf16, kind="ExternalInput").ap()
    wqkT = nc.dram_tensor("wqkT", [C, FQK], bf16, kind="ExternalInput").ap()
    wvT = nc.dram_tensor("wvT", [C, FV], bf16, kind="ExternalInput").ap()
    wpT = nc.dram_tensor("wpT", [C, FV], bf16, kind="ExternalInput").ap()
    bqk = nc.dram_tensor("bqk", [FQK, 1], f32, kind="ExternalInput").ap()
    bv = nc.dram_tensor("bv", [1, FV], bf16, kind="ExternalInput").ap()
    bp = nc.dram_tensor("bp", [1, FV], bf16, kind="ExternalInput").ap()
    maskb = nc.dram_tensor("maskb", [128, 128], bf16, kind="ExternalInput").ap()
    out = nc.dram_tensor("out", [T, FV], f32, kind="ExternalOutput").ap()

    xTr = xT.rearrange("(n p) t -> p n t", p=128)
    wqkr = wqkT.rearrange("(n p) f -> p n f", p=128)
    wvr = wvT.rearrange("(n p) f -> p n f", p=128)
    wpr = wpT.rearrange("(n p) f -> p n f", p=128)

    with tile.TileContext(nc) as tc:
        with (
            tc.tile_pool(name="persist", bufs=1) as pp,
            tc.tile_pool(name="work", bufs=4) as wp,
            tc.tile_pool(name="attT", bufs=6) as ap_pool,
            tc.tile_pool(name="outsb", bufs=3) as op,
            tc.tile_pool(name="ps_s", bufs=4, space="PSUM") as ps_s,
            tc.tile_pool(name="ps_y", bufs=2, space="PSUM") as ps_y,
            tc.tile_pool(name="ps_mm", bufs=2, space="PSUM") as ps_mm,
            tc.tile_pool(name="dram", bufs=1, space="DRAM") as dram,
        ):
            # ---- DMA loads, ordered so qkT(0)'s operands land first ----
            bqk_s = pp.tile([128, 4], f32, tag="bqk")
            nc.sync.dma_start(bqk_s[:], bqk.rearrange("(n p) o -> p (n o)", p=128))
            mask_s = pp.tile([128, 128], bf16, tag="mask")
            nc.sync.dma_start(mask_s[:], maskb[:])

            wqk_s = pp.tile([128, CT, FQK], bf16, tag="wqk")
            xT_s = pp.tile([128, CT, T], bf16, tag="xT")
            wv_s = pp.tile([128, CT, FV], bf16, tag="wv")
            wp_s = pp.tile([128, CT, FV], bf16, tag="wp")
            for ci in range(CT):
                nc.sync.dma_start(wqk_s[:, ci, :], wqkr[:, ci, :])
                nc.sync.dma_start(xT_s[:, ci, 0:512], xTr[:, ci, 0:512])
            for t5x in range(1, TC5):
                for ci in range(CT):
                    nc.sync.dma_start(xT_s[:, ci, t5x * 512:(t5x + 1) * 512],
                                      xTr[:, ci, t5x * 512:(t5x + 1) * 512])
            for ci in range(CT):
                nc.sync.dma_start(wv_s[:, ci, :], wvr[:, ci, :])
            nc.sync.dma_start(wp_s[:], wpr[:])
            bv_s = pp.tile([1, FV], bf16, tag="bv")
            nc.sync.dma_start(bv_s[:], bv[:])
            bp_s = pp.tile([1, FV], bf16, tag="bp")
            nc.sync.dma_start(bp_s[:], bp[:])

            ones16 = pp.tile([1, 128], bf16, tag="ones16")
            nc.vector.memset(ones16[:], 1.0)

            # ---- qkv producers ----
            qkT_s = [pp.tile([128, T], bf16, tag=f"qkT{fc}", name=f"qkT{fc}")
                     for fc in range(4)]
            vaug = pp.tile([128, SB, HPC * 65], bf16, tag="vaug")
            nc.vector.memset(vaug[:], 1.0)
            bv_bc = pp.tile([128, FV], f32, tag="bv_bc")
            bp_bc = pp.tile([128, FV], f32, tag="bp_bc")

            def qkT_chunk(t5):
                for fc in range(4):
                    ps = ps_s.tile([128, 512], f32, tag="s", name="ps_qkv")
                    for ci in range(CT):
                        nc.tensor.matmul(
                            ps[:],
                            wqk_s[:, ci, fc * 128:(fc + 1) * 128],
                            xT_s[:, ci, t5 * 512:(t5 + 1) * 512],
                            start=(ci == 0), stop=(ci == CT - 1),
                        )
                    nc.vector.tensor_scalar_add(
                        qkT_s[fc][:, t5 * 512:(t5 + 1) * 512], ps[:],
                        bqk_s[:, fc:fc + 1],
                    )

            def bias_bc(row, bc_t):
                psb = ps_mm.tile([128, FV], f32, tag="mm", name="ps_bias")
                nc.tensor.matmul(psb[:], ones16[0:1, :], row[0:1, :],
                                 start=True, stop=True)
                nc.vector.tensor_copy(bc_t[:], psb[:])

            def v_chunk(t5):
                for tb in range(4 * t5, 4 * t5 + 4):
                    ps = ps_mm.tile([128, FV], f32, tag="mm", name="ps_v")
                    for ci in range(CT):
                        nc.tensor.matmul(
                            ps[:],
                            xT_s[:, ci, tb * 128:(tb + 1) * 128],
                            wv_s[:, ci, :],
                            start=(ci == 0), stop=(ci == CT - 1),
                        )
                    dst = vaug[:, tb, :].rearrange("p (h x) -> p h x", h=HPC)[:, :, 0:64]
                    src = ps[:].rearrange("p (h x) -> p h x", h=HPC)
                    bias = bv_bc[:].rearrange("p (h x) -> p h x", h=HPC)
                    nc.vector.scalar_tensor_tensor(
                        dst, src, 1.0, bias,
                        op0=mybir.AluOpType.mult, op1=mybir.AluOpType.add,
                    )

            # ---- attention: per-head pipelined QK -> exp -> AV ----
            # norm (den bcast matmul etc.) for head h is deferred into head
            # h+1's QK stream so the PE queue never stalls on the DVE chain.
            ag_in, ag_out, yf = {}, {}, {}
            for t5 in range(TC5):
                ag_in[t5] = dram.tile([256, 512], bf16, tag=f"agin{t5}",
                                      name=f"agin{t5}")
                ag_out[t5] = dram.tile([1024, 512], bf16, tag=f"agout{t5}",
                                       name=f"agout{t5}")
            pending = []

            def flush_pending():
                while pending:
                    pending.pop(0)()

            def att_chunk(t5):
                live = 4 * (t5 + 1)
                yf[t5] = pp.tile([128, 2, CT // 2, 512], bf16, tag="yf", bufs=2,
                                 name=f"yf{t5}")
                for head in range(HPC):
                    pair, hh = head // 2, head % 2
                    q_fc, k_fc = pair, 2 + pair
                    lo, hi = 64 * hh, 64 * (hh + 1)
                    yps = ps_y.tile([65, 512], f32, tag="y", name="yps")
                    a_t = [None] * live
                    qlo = [max(0, sb * 128 - t5 * 512) for sb in range(live)]

                    def qk_block(sb):
                        ps = ps_s.tile([128, 512], f32, tag="s", name="ps_s")
                        nc.tensor.matmul(
                            ps[:, qlo[sb]:512],
                            qkT_s[k_fc][lo:hi, sb * 128:(sb + 1) * 128],
                            qkT_s[q_fc][lo:hi,
                                        t5 * 512 + qlo[sb]:(t5 + 1) * 512],
                            start=True, stop=True,
                        )
                        a = ap_pool.tile([128, 512], bf16, tag="attT",
                                         name="attT")
                        nc.scalar.activation(
                            a[:, qlo[sb]:512], ps[:, qlo[sb]:512],
                            mybir.ActivationFunctionType.Exp, scale=SCALE,
                        )
                        off = sb * 128 - t5 * 512
                        if off >= 0:
                            nc.vector.tensor_mul(a[:, off:off + 128],
                                                 a[:, off:off + 128], mask_s[:])
                        a_t[sb] = a

                    def av_block(sb):
                        nc.tensor.matmul(
                            yps[:, qlo[sb]:512],
                            vaug[:, sb, head * 65:(head + 1) * 65],
                            a_t[sb][:, qlo[sb]:512],
                            start=(sb == 0), stop=(sb == live - 1),
                            skip_group_check=True,
                        )

                    for sb in range(live):
                        qk_block(sb)
                        if sb == 2:
                            flush_pending()
                        if sb >= 3:
                            av_block(sb - 3)
                    for sb in range(max(0, live - 3), live):
                        av_block(sb)

                    def norm(yps=yps, t5=t5, pair=pair, hh=hh, head=head):
                        # y / denom (denom accumulated via the ones row 64)
                        den = wp.tile([1, 512], bf16, tag="den", name="den")
                        nc.vector.tensor_copy(den[:], yps[64:65, :])
                        bc = ps_s.tile([128, 512], f32, tag="s", name="bc")
                        nc.tensor.matmul(bc[0:64, :], ones16[0:1, 0:64],
                                         den[:], start=True, stop=True)
                        r = wp.tile([64, 512], f32, tag="recip", name="recip")
                        nc.vector.reciprocal_approx_fast(r[:], bc[0:64, :])
                        yn = wp.tile([64, 512], bf16, tag="yn", name="yn")
                        nc.vector.tensor_mul(yn[:], yps[0:64, :], r[:])
                        nc.sync.dma_start(
                            ag_in[t5][pair * 128 + hh * 64:
                                      pair * 128 + (hh + 1) * 64, :], yn[:])
                        if head == HPC - 1:
                            nc.gpsimd.collective_compute(
                                "AllGather", mybir.AluOpType.bypass,
                                replica_groups=[[0, 1, 2, 3], [4, 5, 6, 7]],
                                ins=[ag_in[t5][:].opt()],
                                outs=[ag_out[t5][:].opt()],
                            )
                            src = ag_out[t5][:].rearrange(
                                "(cc pr p) t -> p pr cc t", p=128, pr=2)
                            for pr in range(2):
                                nc.gpsimd.dma_start(yf[t5][:, pr, :, :],
                                                    src[:, pr, :, :])

                    pending.append(norm)

            def proj_chunk(t5, flush_after=None):
                yft = yf[t5]
                for tq in range(4):
                    if tq == 1 and flush_after:
                        flush_pending()
                    tb = t5 * 4 + tq
                    pso = ps_mm.tile([128, FV], f32, tag="mm", name="ps_o")
                    for ci in range(CT):
                        par, cc = ci % 2, ci // 2
                        nc.tensor.matmul(
                            pso[:],
                            yft[:, par, cc, tq * 128:(tq + 1) * 128],
                            wp_s[:, ci, :],
                            start=(ci == 0), stop=(ci == CT - 1),
                        )
                    osb = op.tile([128, FV], f32, tag="osb", name="osb")
                    nc.vector.tensor_add(osb[:], pso[:], bp_bc[:])
                    nc.sync.dma_start(out[tb * 128:(tb + 1) * 128, :], osb[:])

            # ---- schedule ----
            for t5 in range(TC5):
                qkT_chunk(t5)
            bias_bc(bv_s, bv_bc)
            bias_bc(bp_s, bp_bc)
            for t5 in range(TC5):
                v_chunk(t5)
            att_chunk(3)
            att_chunk(2)
            proj_chunk(3)
            att_chunk(0)
            att_chunk(1)
            proj_chunk(2, flush_after=True)
            proj_chunk(0)
            proj_chunk(1)

    nc.compile()
    return nc


def _shard_inputs(x, w_attn, b_attn, w_proj, b_proj):
    mask = np.zeros((128, 128), dtype=BF16)
    for p in range(128):
        mask[p, p:] = 1.0

    in_maps = []
    for core in range(NCORES):
        b, hg = core // 4, core % 4
        r0 = hg * HPC * D          # first q/k/v row offset within each 1024
        r1 = r0 + HPC * D
        wqk = np.concatenate([w_attn[r0:r1, :], w_attn[C + r0:C + r1, :]], 0)
        in_maps.append({
            "xT": np.ascontiguousarray(x[b].T).astype(BF16),
            "wqkT": np.ascontiguousarray(wqk.T).astype(BF16),
            "wvT": np.ascontiguousarray(w_attn[2 * C + r0:2 * C + r1, :].T).astype(BF16),
            "wpT": np.ascontiguousarray(w_proj[r0:r1, :].T).astype(BF16),
            "bqk": np.concatenate([b_attn[r0:r1], b_attn[C + r0:C + r1]])
                     .reshape(FQK, 1).astype(np.float32),
            "bv": b_attn[2 * C + r0:2 * C + r1].reshape(1, FV).astype(BF16),
            "bp": b_proj[r0:r1].reshape(1, FV).astype(BF16),
            "maskb": mask,
        })
    return in_maps


def kernel(x, w_attn, b_attn, w_proj, b_proj, _trace=False, _trace_kwargs=None):
    x = np.asarray(x, dtype=np.float32)
    w_attn = np.asarray(w_attn, dtype=np.float32)
    b_attn = np.asarray(b_attn, dtype=np.float32)
    w_proj = np.asarray(w_proj, dtype=np.float32)
    b_proj = np.asarray(b_proj, dtype=np.float32)

    if "nc" not in _CACHE:
        _CACHE["nc"] = _build_kernel()
    nc = _CACHE["nc"]

    in_maps = _shard_inputs(x, w_attn, b_attn, w_proj, b_proj)
    res = run_bass_kernel_spmd(nc, in_maps, core_ids=list(range(NCORES)),
                               trace=_trace, **(_trace_kwargs or {}))
    _CACHE["last_result"] = res

    out = np.empty((B, T, C), dtype=np.float32)
    for core in range(NCORES):
        b, hg = core // 4, core % 4
        out[b, :, hg * FV:(hg + 1) * FV] = res.results[core]["out"]
    return out
